# revision 75
# baseline (speedup 1.0000x reference)
"""MultiHeadAttention TRN2 kernel (B=2, S=2048, D=1024, H=16).

The reference reshapes (B,S,D)->(B*H,S,dk) contiguously (no transpose), which
makes attention local to blocks of 128 consecutive rows of the flattened
(4096, 1024) activations.  Shard 512 rows per core across 8 cores; each core
runs 4 independent 128-row attention groups plus its slice of the QKV/output
projections.

Host path: the jitted shard_map runner is built once; weight tensors are
device-resident across calls; only q/k/v activations (bf16) are shipped per
call and the bf16 output is fetched back.
"""

import os
import sys
import time
import types
from contextlib import ExitStack

import ml_dtypes
import numpy as np

try:
    import concourse.bacc as bacc
except ImportError:
    sys.path.insert(0, "/opt/trn_rl_repo")
    import concourse.bacc as bacc

import concourse.mybir as mybir
import concourse.tile as tile

F32 = mybir.dt.float32
F32R = mybir.dt.float32r
BF16 = mybir.dt.bfloat16
F16 = mybir.dt.float16

N_CORES = 8
RPC = 512          # rows per core of the (4096, 1024) flattened activations
D = 1024
NG = 4             # 128-row attention groups per core

_CACHE = {}
LAST_EXEC_NS = None


def _install_ntff_hook():
    """Recreate the missing antenv.axon_hooks module so trace=True works."""
    if "antenv.axon_hooks" in sys.modules:
        return
    try:
        from trn_agent_boot.trn_boot import _ntff_profile_via_ctypes

        hook = _ntff_profile_via_ctypes("/opt/axon/libaxon_pjrt.so")
        mod = types.ModuleType("antenv.axon_hooks")
        mod.get_axon_ntff_profile_hook = lambda: hook
        import antenv

        sys.modules["antenv.axon_hooks"] = mod
        antenv.axon_hooks = mod
    except Exception:
        pass


def _build():
    nc = bacc.Bacc(None, target_bir_lowering=False, debug=False)
    with tile.TileContext(nc) as tc:
        es = ExitStack()
        with es:
            dram = es.enter_context(tc.tile_pool(name="dram", bufs=1, space="DRAM"))
            xqt_d = dram.tile([128, 8, RPC], BF16, kind="ExternalInput", name="xqt", uniquify=False)
            xkt_d = dram.tile([128, 8, RPC], BF16, kind="ExternalInput", name="xkt", uniquify=False)
            xvt_d = dram.tile([128, 8, RPC], BF16, kind="ExternalInput", name="xvt", uniquify=False)
            wqt_d = dram.tile([128, 8, D], BF16, kind="ExternalInput", name="wqt", uniquify=False)
            wkt_d = dram.tile([128, 8, D], BF16, kind="ExternalInput", name="wkt", uniquify=False)
            wvt_d = dram.tile([128, 8, D], BF16, kind="ExternalInput", name="wvt", uniquify=False)
            wot_d = dram.tile([128, 8, D], BF16, kind="ExternalInput", name="wot", uniquify=False)
            bqp_d = dram.tile([128, 8], F32, kind="ExternalInput", name="bqp", uniquify=False)
            bqr_d = dram.tile([1, D], BF16, kind="ExternalInput", name="bqr", uniquify=False)
            bkp_d = dram.tile([128, 8], F32, kind="ExternalInput", name="bkp", uniquify=False)
            bkr_d = dram.tile([1, D], BF16, kind="ExternalInput", name="bkr", uniquify=False)
            bv_d = dram.tile([1, D], BF16, kind="ExternalInput", name="bv", uniquify=False)
            y_d = dram.tile([RPC, D], BF16, kind="ExternalOutput", name="y", uniquify=False)

            constp = es.enter_context(tc.tile_pool(name="const", bufs=1))
            bqp_sb = constp.tile([128, 8], F32)
            bqr_sb = constp.tile([1, D], BF16)
            bkp_sb = constp.tile([128, 8], F32)
            bkr_sb = constp.tile([1, D], BF16)
            bv_sb = constp.tile([1, D], BF16)
            ones_b = constp.tile([1, 512], BF16)
            nc.gpsimd.memset(ones_b[:, :], 1.0)

            att_cm = tc.tile_pool(name="att", bufs=1)
            attp = att_cm.__enter__()
            att2 = attp.tile([128, 8, RPC], BF16)

            qkv_cm = tc.tile_pool(name="qkv", bufs=1)
            qkvp = qkv_cm.__enter__()
            # qt: head h replicated on both partition halves.
            # slot s in 0..7 -> head 2s ; slot 8+s -> head 2s+1
            qt_sb = qkvp.tile([128, 16, RPC], BF16)
            # kt: slot m holds head 2m on partitions 0:64, head 2m+1 on 64:128
            kt_sb = qkvp.tile([128, 8, RPC], BF16)
            # v: [keys t, group, head, dk + ones col]
            v_sb = qkvp.tile([128, NG, 16, 65], BF16)
            for j in range(NG):
                nc.gpsimd.memset(v_sb[:, j, :, 64:65], 1.0)

            # V weights/activations stay resident through attention
            wpv_cm = tc.tile_pool(name="wpv", bufs=1)
            wpv = wpv_cm.__enter__()
            wv_sb = wpv.tile([128, 8, D], BF16)
            xv_sb = wpv.tile([128, 8, RPC], BF16)

            # ---------- Q/K projections + V-proj for group 0 ----------
            with tc.tile_pool(name="wpqk", bufs=1) as wp, \
                 tc.tile_pool(name="psA", bufs=1, space="PSUM") as psA:
                wq_sb = wp.tile([128, 8, D], BF16)
                wk_sb = wp.tile([128, 8, D], BF16)
                xq_sb = wp.tile([128, 8, RPC], BF16)
                xk_sb = wp.tile([128, 8, RPC], BF16)
                # Input loads: biases first (tiny), then per-kc w/x pairs in
                # projection consumption order.  Splitting across engine
                # queues measured no better than this (phase A is paced by
                # per-queue DMA bandwidth either way).
                # first Q-proj matmul only needs wq[:,0,0:128] + xq[:,0,:] —
                # split those off so it can start ~5us earlier
                nc.sync.dma_start(wq_sb[:, 0, 0:128], wqt_d[:, 0, 0:128])
                nc.scalar.dma_start(xq_sb[:, 0, :], xqt_d[:, 0, :])
                nc.sync.dma_start(wq_sb[:, 0, 128:1024], wqt_d[:, 0, 128:1024])
                for kc in range(1, 8):
                    nc.sync.dma_start(wq_sb[:, kc, :], wqt_d[:, kc, :])
                    nc.scalar.dma_start(xq_sb[:, kc, :], xqt_d[:, kc, :])
                nc.scalar.dma_start(bqp_sb[:, :], bqp_d[:, :])
                nc.scalar.dma_start(bqr_sb[:, :], bqr_d[:, :])
                nc.scalar.dma_start(bkp_sb[:, :], bkp_d[:, :])
                nc.scalar.dma_start(bkr_sb[:, :], bkr_d[:, :])
                nc.scalar.dma_start(bv_sb[:, :], bv_d[:, :])
                nc.gpsimd.dma_start(xv_sb[:, 0:8, 0:128], xvt_d[:, 0:8, 0:128])
                for kc in range(8):
                    nc.gpsimd.dma_start(wv_sb[:, kc, :], wvt_d[:, kc, :])
                for kc in range(8):
                    nc.scalar.dma_start(wk_sb[:, kc, :], wkt_d[:, kc, :])
                    nc.sync.dma_start(xk_sb[:, kc, :], xkt_d[:, kc, :])
                nc.gpsimd.dma_start(xv_sb[:, 0:8, 128:512], xvt_d[:, 0:8, 128:512])

                # Q projection, kc-major: 8 concurrent PSUM accumulators so
                # compute pipelines with the per-kc DMA arrival of wq instead
                # of each column waiting for the whole weight tensor.
                # ps holds heads (2cc2, 2cc2+1) on the two partition halves;
                # write into qt slots and replicate.
                qps = psA.tile([128, 8, 512], F32)
                for kc in range(7):
                    for cc2 in range(8):
                        nc.tensor.matmul(qps[:, cc2, :],
                                         wq_sb[:, kc, 128 * cc2:128 * cc2 + 128],
                                         xq_sb[:, kc, :],
                                         start=(kc == 0), stop=False,
                                         skip_group_check=True)
                # last contraction step per column followed immediately by its
                # epilogue so the DVE chain overlaps the remaining matmuls
                for cc2 in range(8):
                    nc.tensor.matmul(qps[:, cc2, :],
                                     wq_sb[:, 7, 128 * cc2:128 * cc2 + 128],
                                     xq_sb[:, 7, :],
                                     start=False, stop=True,
                                     skip_group_check=True)
                    nc.vector.tensor_scalar_add(qt_sb[0:64, cc2, :],
                                                qps[0:64, cc2, :],
                                                bqp_sb[0:64, cc2:cc2 + 1])
                    nc.vector.tensor_scalar_add(qt_sb[64:128, 8 + cc2, :],
                                                qps[64:128, cc2, :],
                                                bqp_sb[64:128, cc2:cc2 + 1])
                    nc.sync.dma_start(qt_sb[64:128, cc2, :], qt_sb[0:64, cc2, :])
                    nc.gpsimd.dma_start(qt_sb[0:64, 8 + cc2, :], qt_sb[64:128, 8 + cc2, :])

                # V-proj group 0 (its inputs land early on the gpsimd queue)
                vps = qps
                for h in range(2):
                    for kc in range(8):
                        nc.tensor.matmul(vps[:, h, :],
                                         xv_sb[:, kc, 0:128],
                                         wv_sb[:, kc, 512 * h:512 * h + 512],
                                         start=(kc == 0), stop=False,
                                         skip_group_check=True)
                    nc.tensor.matmul(vps[:, h, :],
                                     ones_b[0:1, 0:128],
                                     bv_sb[0:1, 512 * h:512 * h + 512],
                                     start=False, stop=True,
                                     skip_group_check=True)
                    nc.vector.tensor_copy(v_sb[:, 0, 8 * h:8 * h + 8, 0:64],
                                          vps[:, h, :])

                # K projection, kc-major; ps layout == kt slot layout.
                kps = qps
                for kc in range(7):
                    for cc2 in range(8):
                        nc.tensor.matmul(kps[:, cc2, :],
                                         wk_sb[:, kc, 128 * cc2:128 * cc2 + 128],
                                         xk_sb[:, kc, :],
                                         start=(kc == 0), stop=False,
                                         skip_group_check=True)
                # Last contraction step per column, staging split across the
                # vector AND scalar engines so the trailing chain after K's
                # last matmul is ~1.5us (a single-engine chain of 8 ops trails
                # ~4.3us, which also re-throttles HAM before attention).
                for cc2 in range(8):
                    if cc2 % 2 == 0:
                        nc.tensor.matmul(kps[:, cc2, :],
                                         wk_sb[:, 7, 128 * cc2:128 * cc2 + 128],
                                         xk_sb[:, 7, :],
                                         start=False, stop=True,
                                         skip_group_check=True)
                        nc.vector.tensor_scalar_add(kt_sb[:, cc2, :],
                                                    kps[:, cc2, :],
                                                    bkp_sb[:, cc2:cc2 + 1])
                    else:
                        nc.tensor.matmul(kps[:, cc2, :],
                                         wk_sb[:, 7, 128 * cc2:128 * cc2 + 128],
                                         xk_sb[:, 7, :],
                                         start=False, stop=False,
                                         skip_group_check=True)
                        nc.tensor.matmul(kps[:, cc2, :],
                                         bkr_sb[0:1, 128 * cc2:128 * cc2 + 128],
                                         ones_b[0:1, 0:512],
                                         start=False, stop=True,
                                         skip_group_check=True)
                        nc.scalar.activation(kt_sb[:, cc2, :],
                                             kps[:, cc2, :],
                                             mybir.ActivationFunctionType.Copy)

            # ---------- attention with interleaved V-proj / out-proj filler ----------
            with tc.tile_pool(name="wo", bufs=1) as wop, \
                 tc.tile_pool(name="obp", bufs=2) as obp, \
                 tc.tile_pool(name="expp", bufs=3) as expp, \
                 tc.tile_pool(name="smp", bufs=2) as smp, \
                 tc.tile_pool(name="pqk", bufs=2, space="PSUM") as pqk, \
                 tc.tile_pool(name="pav", bufs=2, space="PSUM") as pav, \
                 tc.tile_pool(name="pbc", bufs=1, space="PSUM") as pbc, \
                 tc.tile_pool(name="psvo", bufs=1, space="PSUM") as psvo:
                wot_sb = wop.tile([128, 8, D], BF16)
                for kc in range(8):
                    nc.sync.dma_start(wot_sb[:, kc, :], wot_d[:, kc, :])

                def vproj_units(j):
                    for h in range(2):
                        ps = psvo.tile([128, 512], F32)
                        for kc in range(8):
                            nc.tensor.matmul(ps[:, :],
                                             xv_sb[:, kc, 128 * j:128 * j + 128],
                                             wv_sb[:, kc, 512 * h:512 * h + 512],
                                             start=(kc == 0), stop=False)
                            yield
                        nc.tensor.matmul(ps[:, :],
                                         ones_b[0:1, 0:128],
                                         bv_sb[0:1, 512 * h:512 * h + 512],
                                         start=False, stop=True)
                        nc.vector.tensor_copy(v_sb[:, j, 8 * h:8 * h + 8, 0:64], ps[:, :])
                        yield

                def op_units(jj):
                    for h in range(2):
                        ps = psvo.tile([128, 512], F32)
                        for cc2 in range(8):
                            nc.tensor.matmul(
                                ps[:, :],
                                att2[:, cc2, 128 * jj:128 * jj + 128],
                                wot_sb[:, cc2, 512 * h:512 * h + 512],
                                start=(cc2 == 0), stop=(cc2 == 7))
                            yield
                        ob = obp.tile([128, 512], BF16)
                        nc.vector.tensor_copy(ob[:, :], ps[:, :])
                        nc.gpsimd.dma_start(y_d[128 * jj:128 * jj + 128,
                                                512 * h:512 * h + 512],
                                            ob[:, :])
                        yield

                # Tail overlap: group 3's h=0 out-proj chain starts with the
                # cc2 0..3 partials (they only need the sb0/sb2 blocks of
                # att2, complete before sb3's m-loop runs).  The filler slot
                # count per sb block is 16, so 32 no-op slots delay these
                # partials to (j=3, sb=3).
                ps3 = [None]

                def op3_part1():
                    ps = psvo.tile([128, 512], F32)
                    ps3[0] = ps
                    for cc2 in range(4):
                        nc.tensor.matmul(
                            ps[:, :],
                            att2[:, cc2, 384:512],
                            wot_sb[:, cc2, 0:512],
                            start=(cc2 == 0), stop=False)
                        yield

                qe3 = [None]

                def op3_h1a():
                    # Fill the final normalize window: h=1 partials for cc2
                    # 0..3 (sb0/sb2 data) in the last m-loop qk tile (dead
                    # after its exp), plus the lower-half (c=64, sb1 data)
                    # partial contractions of cc2 4..7 for both h chains —
                    # only the sb3 upper halves remain dependent on the last
                    # att2 store.
                    qe = qe3[0]
                    ps = ps3[0]
                    for cc2 in range(4):
                        nc.tensor.matmul(
                            qe[:, 0:512],
                            att2[:, cc2, 384:512],
                            wot_sb[:, cc2, 512:1024],
                            start=(cc2 == 0), stop=False,
                            skip_group_check=True)
                    for cc2 in range(4, 8):
                        nc.tensor.matmul(
                            ps[:, :],
                            att2[0:64, cc2, 384:512],
                            wot_sb[0:64, cc2, 0:512],
                            start=False, stop=False,
                            skip_group_check=True)
                        nc.tensor.matmul(
                            qe[:, 0:512],
                            att2[0:64, cc2, 384:512],
                            wot_sb[0:64, cc2, 512:1024],
                            start=False, stop=False,
                            skip_group_check=True)

                def op3_rest():
                    ps = ps3[0]
                    qe = qe3[0]
                    for cc2 in range(4, 8):
                        nc.tensor.matmul(
                            ps[:, :],
                            att2[64:128, cc2, 384:512],
                            wot_sb[64:128, cc2, 0:512],
                            start=False, stop=(cc2 == 7),
                            skip_group_check=True)
                        yield
                    ob = obp.tile([128, 512], BF16)
                    nc.vector.tensor_copy(ob[:, :], ps[:, :])
                    nc.gpsimd.dma_start(y_d[384:512, 0:512], ob[:, :])
                    yield
                    for cc2 in range(4, 8):
                        nc.tensor.matmul(
                            qe[:, 0:512],
                            att2[64:128, cc2, 384:512],
                            wot_sb[64:128, cc2, 512:1024],
                            start=False, stop=(cc2 == 7),
                            skip_group_check=True)
                        yield
                    ob = obp.tile([128, 512], BF16)
                    nc.vector.tensor_copy(ob[:, :], qe[:, 0:512])
                    nc.gpsimd.dma_start(y_d[384:512, 512:1024], ob[:, :])
                    yield

                import itertools
                fill = {
                    0: itertools.chain(vproj_units(1)),
                    1: itertools.chain(vproj_units(2), op_units(0)),
                    2: itertools.chain(vproj_units(3), op_units(1)),
                    3: itertools.chain(op_units(2), itertools.repeat(None, 32),
                                       op3_part1()),
                }

                def epilogue_finish(pj, psb, pav_t, prcb, pbc_t):
                    # bc broadcast + normalize + att2 store for a completed
                    # block; deferred into the next block's ACT-latency shadow
                    # so the tensor queue never stalls on the DVE recip chain.
                    nc.tensor.matmul(pbc_t[:, :],
                                     ones_b[0:1, 0:64],
                                     prcb[:, :],
                                     start=True, stop=True, skip_group_check=True)
                    ar = smp.tile([64, 512], F32)
                    nc.vector.tensor_copy(ar[:, :], pav_t[0:64, :])
                    sm2 = smp.tile([64, 512], BF16)
                    nc.vector.tensor_mul(sm2[:, :], ar[:, :], pbc_t[:, :])
                    # heads of this block: sb 0,1 -> even heads -> partitions
                    # 0:64 of att2 (DVE copy); sb 2,3 -> odd heads -> 64:128 (DMA)
                    c2lo = 4 * (psb % 2)
                    if psb < 2:
                        nc.vector.tensor_copy(
                            att2[0:64, c2lo:c2lo + 4, 128 * pj:128 * pj + 128],
                            sm2[:, :])
                    else:
                        nc.gpsimd.dma_start(
                            att2[64:128, c2lo:c2lo + 4, 128 * pj:128 * pj + 128],
                            sm2[:, :])

                pending = None
                for j in range(NG):
                    gen = fill[j]
                    for sb in range(4):
                        av = pav.tile([65, 512], F32)
                        for m in range(8):
                            qk = pqk.tile([128, 1024], F32)
                            if j == 3 and sb == 3 and m == 7:
                                qe3[0] = qk
                            nc.tensor.matmul(
                                qk[:, 0:512],
                                kt_sb[0:64, m, 128 * j:128 * j + 128],
                                qt_sb[0:64, 4 * sb:4 * sb + 4, 128 * j:128 * j + 128],
                                start=True, stop=True, skip_group_check=True,
                                tile_position=(0, 0))
                            nc.tensor.matmul(
                                qk[:, 512:1024],
                                kt_sb[64:128, m, 128 * j:128 * j + 128],
                                qt_sb[64:128, 4 * sb:4 * sb + 4, 128 * j:128 * j + 128],
                                start=True, stop=True, skip_group_check=True,
                                tile_position=(64, 0))
                            ex = expp.tile([128, 1024], BF16)
                            nc.scalar.activation(ex[:, :], qk[:, :],
                                                 mybir.ActivationFunctionType.Exp,
                                                 bias=0.0, scale=0.125)
                            if m == 0 and pending is not None:
                                epilogue_finish(*pending)
                                pending = None
                            for i in range(2):
                                ct = 2 * m + i
                                nc.tensor.matmul(av[:, :],
                                                 v_sb[:, j, ct, :],
                                                 ex[:, 512 * i:512 * i + 512],
                                                 start=(ct == 0), stop=(ct == 15),
                                                 skip_group_check=True)
                            for _ in range(2):
                                try:
                                    next(gen)
                                except StopIteration:
                                    break
                        ds = smp.tile([1, 512], F32)
                        nc.vector.tensor_copy(ds[:, :], av[64:65, :])
                        rc = smp.tile([1, 512], F32)
                        nc.vector.reciprocal_approx_fast(rc[:, :], ds[:, :])
                        rcb = smp.tile([1, 512], BF16)
                        nc.vector.tensor_copy(rcb[:, :], rc[:, :])
                        bc = pbc.tile([64, 512], F32)
                        pending = (j, sb, av, rcb, bc)
                    # pending carries across group boundaries (the next
                    # group's op fillers don't read this group's last att2
                    # block until well after its deferred store lands); only
                    # the final block drains here.
                    if j == 3:
                        op3_h1a()
                        epilogue_finish(*pending)
                        pending = None
                    for _ in gen:
                        pass

                for _ in op3_rest():
                    pass

            wpv_cm.__exit__(None, None, None)
            qkv_cm.__exit__(None, None, None)
            att_cm.__exit__(None, None, None)

    nc.compile()
    return nc


def _make_runner(nc):
    """Build the jitted shard_map runner once (mirrors run_bass_via_pjrt)."""
    import jax
    import jax.numpy as jnp
    from jax.sharding import Mesh, PartitionSpec, NamedSharding
    from jax.experimental.shard_map import shard_map
    from concourse import bass2jax
    from concourse.bass2jax import _bass_exec_p, install_neuronx_cc_hook

    install_neuronx_cc_hook()

    part_tensor_name = nc.partition_id_tensor.name if nc.partition_id_tensor else None
    in_names = []
    out_names = []
    out_avals = []
    zero_shapes = []
    for alloc in nc.m.functions[0].allocations:
        if not isinstance(alloc, mybir.MemoryLocationSet):
            continue
        name = alloc.memorylocations[0].name
        if alloc.kind == "ExternalInput":
            if name != part_tensor_name:
                in_names.append(name)
        elif alloc.kind == "ExternalOutput":
            shape = tuple(alloc.tensor_shape)
            dtype = mybir.dt.np(alloc.dtype)
            out_names.append(name)
            out_avals.append(jax.core.ShapedArray(shape, dtype))
            zero_shapes.append((shape, dtype))
    n_params = len(in_names)
    all_names = list(in_names) + list(out_names)
    part_name = nc.partition_id_tensor.name if nc.partition_id_tensor else None
    if part_name is not None:
        all_names.append(part_name)

    def _body(*args):
        operands = list(args)
        if part_name is not None:
            operands.append(bass2jax.partition_id_tensor())
        outs = _bass_exec_p.bind(
            *operands,
            out_avals=tuple(out_avals),
            in_names=tuple(all_names),
            out_names=tuple(out_names),
            lowering_input_output_aliases=(),
            sim_require_finite=True,
            sim_require_nnan=True,
            nc=nc,
        )
        return tuple(outs)

    devices = jax.devices()[:N_CORES]
    mesh = Mesh(np.asarray(devices), ("core",))
    spec = PartitionSpec("core")
    n_outs = len(out_names)
    donate = tuple(range(n_params, n_params + n_outs))
    sharded = jax.jit(
        shard_map(_body, mesh=mesh,
                  in_specs=(spec,) * (n_params + n_outs),
                  out_specs=(spec,) * n_outs,
                  check_rep=False),
        donate_argnums=donate,
        keep_unused=True,
    )

    def _zeros():
        return tuple(
            jnp.zeros((N_CORES * s[0],) + tuple(s[1:]), dt)
            for s, dt in zero_shapes
        )

    zeros_fn = jax.jit(
        _zeros,
        out_shardings=tuple(NamedSharding(mesh, spec) for _ in zero_shapes),
    )

    return {
        "in_names": in_names,
        "out_names": out_names,
        "sharded": sharded,
        "zeros_fn": zeros_fn,
        "mesh": mesh,
        "spec": spec,
    }


def _tr_w(W):
    # [1024, 1024] -> [128, 8, 1024]: [p, kc, f] = W[f, 128*kc+p]
    return np.ascontiguousarray(W.T.reshape(8, 128, D).transpose(1, 0, 2))


def _prep_weights(Wq, bq, Wk, bk, Wv, bv, Wo, bo):
    bf = ml_dtypes.bfloat16
    wqt = _tr_w(np.asarray(Wq, np.float32)).astype(bf)
    wkt = _tr_w(np.asarray(Wk, np.float32)).astype(bf)
    wvt = _tr_w(np.asarray(Wv, np.float32)).astype(bf)
    wot = _tr_w(np.asarray(Wo, np.float32)).astype(bf)
    bqp = np.ascontiguousarray(np.asarray(bq, np.float32).reshape(8, 128).T)
    bqr = np.asarray(bq, np.float32).reshape(1, D).astype(bf)
    bkp = np.ascontiguousarray(np.asarray(bk, np.float32).reshape(8, 128).T)
    bkr = np.asarray(bk, np.float32).reshape(1, D).astype(bf)
    bv2 = np.asarray(bv, np.float32).reshape(1, D).astype(bf)
    return {"wqt": wqt, "wkt": wkt, "wvt": wvt, "wot": wot,
            "bqp": bqp, "bqr": bqr, "bkp": bkp, "bkr": bkr, "bv": bv2}


def _global_x(x_bf):
    # [4096, 1024] bf16 -> global [8*128, 8, 512]:
    # G[c*128+p, kc, r] = x[512*c + r, 128*kc + p]
    return np.ascontiguousarray(
        x_bf.reshape(8, 512, 8, 128).transpose(0, 3, 2, 1).reshape(1024, 8, 512))


def kernel(query, key, value, Wq, bq, Wk, bk, Wv, bv, Wo, bo):
    global LAST_EXEC_NS
    import jax
    from jax.sharding import NamedSharding

    if "nc" not in _CACHE:
        _install_ntff_hook()
        _CACHE["nc"] = _build()
        _CACHE["runner"] = _make_runner(_CACHE["nc"])
    nc = _CACHE["nc"]
    run = _CACHE["runner"]

    bf = ml_dtypes.bfloat16
    wkey = hash((np.asarray(Wq, np.float32).tobytes(),
                 np.asarray(Wk, np.float32).tobytes(),
                 np.asarray(Wv, np.float32).tobytes(),
                 np.asarray(Wo, np.float32).tobytes(),
                 np.asarray(bq, np.float32).tobytes(),
                 np.asarray(bk, np.float32).tobytes(),
                 np.asarray(bv, np.float32).tobytes()))
    if _CACHE.get("wkey") != wkey:
        wmap = _prep_weights(Wq, bq, Wk, bk, Wv, bv, Wo, bo)
        sharding = NamedSharding(run["mesh"], run["spec"])
        dev_w = {}
        for name, arr in wmap.items():
            g = np.concatenate([arr] * N_CORES, axis=0)
            dev_w[name] = jax.device_put(g, sharding)
        _CACHE["dev_w"] = dev_w
        _CACHE["wkey"] = wkey

    dev_w = _CACHE["dev_w"]
    sharding = NamedSharding(run["mesh"], run["spec"])

    def run_once():
        xq = _global_x(np.asarray(query, np.float32).reshape(4096, D).astype(bf))
        xk = _global_x(np.asarray(key, np.float32).reshape(4096, D).astype(bf))
        xv = _global_x(np.asarray(value, np.float32).reshape(4096, D).astype(bf))
        args = []
        for name in run["in_names"]:
            if name == "xqt":
                args.append(jax.device_put(xq, sharding))
            elif name == "xkt":
                args.append(jax.device_put(xk, sharding))
            elif name == "xvt":
                args.append(jax.device_put(xv, sharding))
            else:
                args.append(dev_w[name])
        zeros = run["zeros_fn"]()
        outs = run["sharded"](*args, *zeros)
        return {name: np.asarray(outs[i]) for i, name in enumerate(run["out_names"])}

    # warmup (first call compiles the wrapper executable)
    if "warm" not in _CACHE:
        run_once()
        _CACHE["warm"] = True

    t0 = time.perf_counter()
    res = run_once()
    wall_ns = int((time.perf_counter() - t0) * 1e9)
    _CACHE["wall_ns"] = wall_ns

    # Honest HW execution time: profile once via NTFF (device-side timing).
    if "hw_ns" not in _CACHE:
        _CACHE["hw_ns"] = _measure_hw_ns(query, key, value)
    LAST_EXEC_NS = _CACHE["hw_ns"] if _CACHE["hw_ns"] else wall_ns

    y = res["y"]  # [4096, 1024] bf16
    out = y.astype(np.float32) + np.asarray(bo, np.float32)[None, :]
    return out.reshape(2, 2048, D).astype(np.float32)


def _measure_hw_ns(query, key, value):
    """Run once under NTFF profiling; return on-device NEFF exec time (ns)."""
    try:
        from concourse.bass_utils import run_bass_kernel_spmd

        nc = _CACHE["nc"]
        bf = ml_dtypes.bfloat16
        xq = _global_x(np.asarray(query, np.float32).reshape(4096, D).astype(bf))
        xk = _global_x(np.asarray(key, np.float32).reshape(4096, D).astype(bf))
        xv = _global_x(np.asarray(value, np.float32).reshape(4096, D).astype(bf))
        dev_w = _CACHE["dev_w"]
        # per-core input maps (host copies)
        in_maps = []
        for c in range(N_CORES):
            m = {"xqt": xq[128 * c:128 * c + 128],
                 "xkt": xk[128 * c:128 * c + 128],
                 "xvt": xv[128 * c:128 * c + 128]}
            for name, arr in dev_w.items():
                full = np.asarray(arr)
                per = full.shape[0] // N_CORES
                m[name] = full[per * c:per * c + per]
            in_maps.append(m)
        import shutil
        tdir = "/tmp/ntff_last"
        shutil.rmtree(tdir, ignore_errors=True)
        os.makedirs(tdir, exist_ok=True)
        rr = run_bass_kernel_spmd(nc, in_maps, list(range(N_CORES)), trace=True,
                                  tmpdir=tdir)
        return rr.exec_time_ns
    except Exception:
        return None


# revision 79
# speedup vs baseline: 1.0189x; 1.0189x over previous
"""MultiHeadAttention TRN2 kernel (B=2, S=2048, D=1024, H=16).

The reference reshapes (B,S,D)->(B*H,S,dk) contiguously (no transpose), which
makes attention local to blocks of 128 consecutive rows of the flattened
(4096, 1024) activations.  Shard 512 rows per core across 8 cores; each core
runs 4 independent 128-row attention groups plus its slice of the QKV/output
projections.

Host path: the jitted shard_map runner is built once; weight tensors are
device-resident across calls; only q/k/v activations (bf16) are shipped per
call and the bf16 output is fetched back.
"""

import os
import sys
import time
import types
from contextlib import ExitStack

import ml_dtypes
import numpy as np

try:
    import concourse.bacc as bacc
except ImportError:
    sys.path.insert(0, "/opt/trn_rl_repo")
    import concourse.bacc as bacc

import concourse.mybir as mybir
import concourse.tile as tile

F32 = mybir.dt.float32
F32R = mybir.dt.float32r
BF16 = mybir.dt.bfloat16
F16 = mybir.dt.float16

N_CORES = 8
RPC = 512          # rows per core of the (4096, 1024) flattened activations
D = 1024
NG = 4             # 128-row attention groups per core

_CACHE = {}
LAST_EXEC_NS = None


def _install_ntff_hook():
    """Recreate the missing antenv.axon_hooks module so trace=True works."""
    if "antenv.axon_hooks" in sys.modules:
        return
    try:
        from trn_agent_boot.trn_boot import _ntff_profile_via_ctypes

        hook = _ntff_profile_via_ctypes("/opt/axon/libaxon_pjrt.so")
        mod = types.ModuleType("antenv.axon_hooks")
        mod.get_axon_ntff_profile_hook = lambda: hook
        import antenv

        sys.modules["antenv.axon_hooks"] = mod
        antenv.axon_hooks = mod
    except Exception:
        pass


def _build():
    nc = bacc.Bacc(None, target_bir_lowering=False, debug=False)
    with tile.TileContext(nc) as tc:
        es = ExitStack()
        with es:
            dram = es.enter_context(tc.tile_pool(name="dram", bufs=1, space="DRAM"))
            xqt_d = dram.tile([128, 8, RPC], BF16, kind="ExternalInput", name="xqt", uniquify=False)
            xkt_d = dram.tile([128, 8, RPC], BF16, kind="ExternalInput", name="xkt", uniquify=False)
            xvt_d = dram.tile([128, 8, RPC], BF16, kind="ExternalInput", name="xvt", uniquify=False)
            wqt_d = dram.tile([128, 8, D], BF16, kind="ExternalInput", name="wqt", uniquify=False)
            wkt_d = dram.tile([128, 8, D], BF16, kind="ExternalInput", name="wkt", uniquify=False)
            wvt_d = dram.tile([128, 8, D], BF16, kind="ExternalInput", name="wvt", uniquify=False)
            wot_d = dram.tile([128, 8, D], BF16, kind="ExternalInput", name="wot", uniquify=False)
            bqp_d = dram.tile([128, 8], F32, kind="ExternalInput", name="bqp", uniquify=False)
            bqr_d = dram.tile([1, D], BF16, kind="ExternalInput", name="bqr", uniquify=False)
            bkp_d = dram.tile([128, 8], F32, kind="ExternalInput", name="bkp", uniquify=False)
            bkr_d = dram.tile([1, D], BF16, kind="ExternalInput", name="bkr", uniquify=False)
            bv_d = dram.tile([1, D], BF16, kind="ExternalInput", name="bv", uniquify=False)
            y_d = dram.tile([RPC, D], BF16, kind="ExternalOutput", name="y", uniquify=False)

            constp = es.enter_context(tc.tile_pool(name="const", bufs=1))
            bqp_sb = constp.tile([128, 8], F32)
            bqr_sb = constp.tile([1, D], BF16)
            bkp_sb = constp.tile([128, 8], F32)
            bkr_sb = constp.tile([1, D], BF16)
            bv_sb = constp.tile([1, D], BF16)
            ones_b = constp.tile([1, 512], BF16)
            nc.gpsimd.memset(ones_b[:, :], 1.0)

            att_cm = tc.tile_pool(name="att", bufs=1)
            attp = att_cm.__enter__()
            att2 = attp.tile([128, 8, RPC], BF16)

            qkv_cm = tc.tile_pool(name="qkv", bufs=1)
            qkvp = qkv_cm.__enter__()
            # qt: head h replicated on both partition halves.
            # slot s in 0..7 -> head 2s ; slot 8+s -> head 2s+1
            qt_sb = qkvp.tile([128, 16, RPC], BF16)
            # kt: slot m holds head 2m on partitions 0:64, head 2m+1 on 64:128
            kt_sb = qkvp.tile([128, 8, RPC], BF16)
            # v: [keys t, group, head, dk + ones col]
            v_sb = qkvp.tile([128, NG, 16, 65], BF16)
            for j in range(NG):
                nc.gpsimd.memset(v_sb[:, j, :, 64:65], 1.0)

            # V weights/activations stay resident through attention
            wpv_cm = tc.tile_pool(name="wpv", bufs=1)
            wpv = wpv_cm.__enter__()
            wv_sb = wpv.tile([128, 8, D], BF16)
            xv_sb = wpv.tile([128, 8, RPC], BF16)

            # ---------- Q/K projections + V-proj for group 0 ----------
            with tc.tile_pool(name="wpqk", bufs=1) as wp, \
                 tc.tile_pool(name="psA", bufs=1, space="PSUM") as psA:
                wq_sb = wp.tile([128, 8, D], BF16)
                wk_sb = wp.tile([128, 8, D], BF16)
                xq_sb = wp.tile([128, 8, RPC], BF16)
                xk_sb = wp.tile([128, 8, RPC], BF16)
                # Input loads: biases first (tiny), then per-kc w/x pairs in
                # projection consumption order.  Splitting across engine
                # queues measured no better than this (phase A is paced by
                # per-queue DMA bandwidth either way).
                # first Q-proj matmul only needs wq[:,0,0:128] + xq[:,0,:] —
                # split those off so it can start ~5us earlier
                nc.sync.dma_start(wq_sb[:, 0, 0:128], wqt_d[:, 0, 0:128])
                nc.scalar.dma_start(xq_sb[:, 0, :], xqt_d[:, 0, :])
                nc.sync.dma_start(wq_sb[:, 0, 128:1024], wqt_d[:, 0, 128:1024])
                for kc in range(1, 8):
                    nc.sync.dma_start(wq_sb[:, kc, :], wqt_d[:, kc, :])
                    nc.scalar.dma_start(xq_sb[:, kc, :], xqt_d[:, kc, :])
                nc.scalar.dma_start(bqp_sb[:, :], bqp_d[:, :])
                nc.scalar.dma_start(bqr_sb[:, :], bqr_d[:, :])
                nc.scalar.dma_start(bkp_sb[:, :], bkp_d[:, :])
                nc.scalar.dma_start(bkr_sb[:, :], bkr_d[:, :])
                nc.scalar.dma_start(bv_sb[:, :], bv_d[:, :])
                nc.gpsimd.dma_start(xv_sb[:, 0:8, 0:128], xvt_d[:, 0:8, 0:128])
                for kc in range(8):
                    nc.gpsimd.dma_start(wv_sb[:, kc, :], wvt_d[:, kc, :])
                for kc in range(8):
                    nc.scalar.dma_start(wk_sb[:, kc, :], wkt_d[:, kc, :])
                    nc.sync.dma_start(xk_sb[:, kc, :], xkt_d[:, kc, :])
                nc.gpsimd.dma_start(xv_sb[:, 0:8, 128:512], xvt_d[:, 0:8, 128:512])

                # Q projection, kc-major: 8 concurrent PSUM accumulators so
                # compute pipelines with the per-kc DMA arrival of wq instead
                # of each column waiting for the whole weight tensor.
                # ps holds heads (2cc2, 2cc2+1) on the two partition halves;
                # write into qt slots and replicate.
                qps = psA.tile([128, 8, 512], F32)
                for kc in range(7):
                    for cc2 in range(8):
                        nc.tensor.matmul(qps[:, cc2, :],
                                         wq_sb[:, kc, 128 * cc2:128 * cc2 + 128],
                                         xq_sb[:, kc, :],
                                         start=(kc == 0), stop=False,
                                         skip_group_check=True)
                # last contraction step per column followed immediately by its
                # epilogue so the DVE chain overlaps the remaining matmuls
                for cc2 in range(8):
                    nc.tensor.matmul(qps[:, cc2, :],
                                     wq_sb[:, 7, 128 * cc2:128 * cc2 + 128],
                                     xq_sb[:, 7, :],
                                     start=False, stop=True,
                                     skip_group_check=True)
                    nc.vector.tensor_scalar_add(qt_sb[0:64, cc2, :],
                                                qps[0:64, cc2, :],
                                                bqp_sb[0:64, cc2:cc2 + 1])
                    nc.vector.tensor_scalar_add(qt_sb[64:128, 8 + cc2, :],
                                                qps[64:128, cc2, :],
                                                bqp_sb[64:128, cc2:cc2 + 1])
                    nc.sync.dma_start(qt_sb[64:128, cc2, :], qt_sb[0:64, cc2, :])
                    nc.gpsimd.dma_start(qt_sb[0:64, 8 + cc2, :], qt_sb[64:128, 8 + cc2, :])

                # V-proj group 0 (its inputs land early on the gpsimd queue)
                vps = qps
                for h in range(2):
                    for kc in range(8):
                        nc.tensor.matmul(vps[:, h, :],
                                         xv_sb[:, kc, 0:128],
                                         wv_sb[:, kc, 512 * h:512 * h + 512],
                                         start=(kc == 0), stop=False,
                                         skip_group_check=True)
                    nc.tensor.matmul(vps[:, h, :],
                                     ones_b[0:1, 0:128],
                                     bv_sb[0:1, 512 * h:512 * h + 512],
                                     start=False, stop=True,
                                     skip_group_check=True)
                    nc.vector.tensor_copy(v_sb[:, 0, 8 * h:8 * h + 8, 0:64],
                                          vps[:, h, :])

                # K projection, kc-major; ps layout == kt slot layout.
                kps = qps
                for kc in range(7):
                    for cc2 in range(8):
                        nc.tensor.matmul(kps[:, cc2, :],
                                         wk_sb[:, kc, 128 * cc2:128 * cc2 + 128],
                                         xk_sb[:, kc, :],
                                         start=(kc == 0), stop=False,
                                         skip_group_check=True)
                # Last contraction step per column, staging split across the
                # vector AND scalar engines so the trailing chain after K's
                # last matmul is ~1.5us (a single-engine chain of 8 ops trails
                # ~4.3us, which also re-throttles HAM before attention).
                for cc2 in range(8):
                    if cc2 % 2 == 0:
                        nc.tensor.matmul(kps[:, cc2, :],
                                         wk_sb[:, 7, 128 * cc2:128 * cc2 + 128],
                                         xk_sb[:, 7, :],
                                         start=False, stop=True,
                                         skip_group_check=True)
                        nc.vector.tensor_scalar_add(kt_sb[:, cc2, :],
                                                    kps[:, cc2, :],
                                                    bkp_sb[:, cc2:cc2 + 1])
                    else:
                        nc.tensor.matmul(kps[:, cc2, :],
                                         wk_sb[:, 7, 128 * cc2:128 * cc2 + 128],
                                         xk_sb[:, 7, :],
                                         start=False, stop=False,
                                         skip_group_check=True)
                        nc.tensor.matmul(kps[:, cc2, :],
                                         bkr_sb[0:1, 128 * cc2:128 * cc2 + 128],
                                         ones_b[0:1, 0:512],
                                         start=False, stop=True,
                                         skip_group_check=True)
                        nc.scalar.activation(kt_sb[:, cc2, :],
                                             kps[:, cc2, :],
                                             mybir.ActivationFunctionType.Copy)

            # ---------- attention with interleaved V-proj / out-proj filler ----------
            with tc.tile_pool(name="wo", bufs=1) as wop, \
                 tc.tile_pool(name="obp", bufs=2) as obp, \
                 tc.tile_pool(name="expp", bufs=3) as expp, \
                 tc.tile_pool(name="smp", bufs=2) as smp, \
                 tc.tile_pool(name="pqk", bufs=2, space="PSUM") as pqk, \
                 tc.tile_pool(name="pav", bufs=2, space="PSUM") as pav, \
                 tc.tile_pool(name="pbc", bufs=1, space="PSUM") as pbc, \
                 tc.tile_pool(name="psvo", bufs=1, space="PSUM") as psvo:
                wot_sb = wop.tile([128, 8, D], BF16)
                for kc in range(8):
                    nc.sync.dma_start(wot_sb[:, kc, :], wot_d[:, kc, :])
                # upper-half Wo rows for cc2 4..7 replicated to partitions
                # 0:64 — lets the final block's last out-proj contractions
                # read sm2 in place instead of waiting for the att2 store.
                wot_lo = wop.tile([64, 4, D], BF16)
                nc.sync.dma_start(wot_lo[:, :, :], wot_sb[64:128, 4:8, :])

                def vproj_units(j):
                    for h in range(2):
                        ps = psvo.tile([128, 512], F32)
                        for kc in range(8):
                            nc.tensor.matmul(ps[:, :],
                                             xv_sb[:, kc, 128 * j:128 * j + 128],
                                             wv_sb[:, kc, 512 * h:512 * h + 512],
                                             start=(kc == 0), stop=False)
                            yield
                        nc.tensor.matmul(ps[:, :],
                                         ones_b[0:1, 0:128],
                                         bv_sb[0:1, 512 * h:512 * h + 512],
                                         start=False, stop=True)
                        nc.vector.tensor_copy(v_sb[:, j, 8 * h:8 * h + 8, 0:64], ps[:, :])
                        yield

                def op_units(jj):
                    for h in range(2):
                        ps = psvo.tile([128, 512], F32)
                        for cc2 in range(8):
                            nc.tensor.matmul(
                                ps[:, :],
                                att2[:, cc2, 128 * jj:128 * jj + 128],
                                wot_sb[:, cc2, 512 * h:512 * h + 512],
                                start=(cc2 == 0), stop=(cc2 == 7))
                            yield
                        ob = obp.tile([128, 512], BF16)
                        nc.vector.tensor_copy(ob[:, :], ps[:, :])
                        nc.gpsimd.dma_start(y_d[128 * jj:128 * jj + 128,
                                                512 * h:512 * h + 512],
                                            ob[:, :])
                        yield

                # Tail overlap: group 3's h=0 out-proj chain starts with the
                # cc2 0..3 partials (they only need the sb0/sb2 blocks of
                # att2, complete before sb3's m-loop runs).  The filler slot
                # count per sb block is 16, so 32 no-op slots delay these
                # partials to (j=3, sb=3).
                ps3 = [None]
                sm3 = [None]

                def op3_part1():
                    ps = psvo.tile([128, 512], F32)
                    ps3[0] = ps
                    for cc2 in range(4):
                        nc.tensor.matmul(
                            ps[:, :],
                            att2[:, cc2, 384:512],
                            wot_sb[:, cc2, 0:512],
                            start=(cc2 == 0), stop=False)
                        yield

                qe3 = [None]

                def op3_h1a():
                    # Fill the final normalize window: h=1 partials for cc2
                    # 0..3 (sb0/sb2 data) in the last m-loop qk tile (dead
                    # after its exp), plus the lower-half (c=64, sb1 data)
                    # partial contractions of cc2 4..7 for both h chains —
                    # only the sb3 upper halves remain dependent on the last
                    # att2 store.
                    qe = qe3[0]
                    ps = ps3[0]
                    for cc2 in range(4):
                        nc.tensor.matmul(
                            qe[:, 0:512],
                            att2[:, cc2, 384:512],
                            wot_sb[:, cc2, 512:1024],
                            start=(cc2 == 0), stop=False,
                            skip_group_check=True)
                    for cc2 in range(4, 8):
                        nc.tensor.matmul(
                            ps[:, :],
                            att2[0:64, cc2, 384:512],
                            wot_sb[0:64, cc2, 0:512],
                            start=False, stop=False,
                            skip_group_check=True)
                        nc.tensor.matmul(
                            qe[:, 0:512],
                            att2[0:64, cc2, 384:512],
                            wot_sb[0:64, cc2, 512:1024],
                            start=False, stop=False,
                            skip_group_check=True)

                def op3_rest():
                    # the last block's contribution is read straight from the
                    # normalized sm2 tile (partitions 0:64) against the
                    # replicated Wo rows — no dependency on the att2 store.
                    ps = ps3[0]
                    qe = qe3[0]
                    sm = sm3[0]
                    for cc2 in range(4, 8):
                        hl = 128 * (cc2 - 4)
                        nc.tensor.matmul(
                            ps[:, :],
                            sm[0:64, hl:hl + 128],
                            wot_lo[0:64, cc2 - 4, 0:512],
                            start=False, stop=(cc2 == 7),
                            skip_group_check=True)
                        yield
                    ob = obp.tile([128, 512], BF16)
                    nc.vector.tensor_copy(ob[:, :], ps[:, :])
                    nc.gpsimd.dma_start(y_d[384:512, 0:512], ob[:, :])
                    yield
                    for cc2 in range(4, 8):
                        hl = 128 * (cc2 - 4)
                        nc.tensor.matmul(
                            qe[:, 0:512],
                            sm[0:64, hl:hl + 128],
                            wot_lo[0:64, cc2 - 4, 512:1024],
                            start=False, stop=(cc2 == 7),
                            skip_group_check=True)
                        yield
                    ob = obp.tile([128, 512], BF16)
                    nc.vector.tensor_copy(ob[:, :], qe[:, 0:512])
                    nc.gpsimd.dma_start(y_d[384:512, 512:1024], ob[:, :])
                    yield

                import itertools
                fill = {
                    0: itertools.chain(vproj_units(1)),
                    1: itertools.chain(vproj_units(2), op_units(0)),
                    2: itertools.chain(vproj_units(3), op_units(1)),
                    3: itertools.chain(op_units(2), itertools.repeat(None, 32),
                                       op3_part1()),
                }

                def epilogue_finish(pj, psb, pav_t, prcb, pbc_t):
                    # bc broadcast + normalize + att2 store for a completed
                    # block; deferred into the next block's ACT-latency shadow
                    # so the tensor queue never stalls on the DVE recip chain.
                    nc.tensor.matmul(pbc_t[:, :],
                                     ones_b[0:1, 0:64],
                                     prcb[:, :],
                                     start=True, stop=True, skip_group_check=True)
                    ar = smp.tile([64, 512], F32)
                    nc.vector.tensor_copy(ar[:, :], pav_t[0:64, :])
                    sm2 = smp.tile([64, 512], BF16)
                    nc.vector.tensor_mul(sm2[:, :], ar[:, :], pbc_t[:, :])
                    if pj == 3 and psb == 3:
                        sm3[0] = sm2
                    # heads of this block: sb 0,1 -> even heads -> partitions
                    # 0:64 of att2 (DVE copy); sb 2,3 -> odd heads -> 64:128 (DMA)
                    c2lo = 4 * (psb % 2)
                    if psb < 2:
                        nc.vector.tensor_copy(
                            att2[0:64, c2lo:c2lo + 4, 128 * pj:128 * pj + 128],
                            sm2[:, :])
                    else:
                        nc.gpsimd.dma_start(
                            att2[64:128, c2lo:c2lo + 4, 128 * pj:128 * pj + 128],
                            sm2[:, :])

                pending = None
                for j in range(NG):
                    gen = fill[j]
                    for sb in range(4):
                        av = pav.tile([65, 512], F32)
                        for m in range(8):
                            qk = pqk.tile([128, 1024], F32)
                            if j == 3 and sb == 3 and m == 7:
                                qe3[0] = qk
                            nc.tensor.matmul(
                                qk[:, 0:512],
                                kt_sb[0:64, m, 128 * j:128 * j + 128],
                                qt_sb[0:64, 4 * sb:4 * sb + 4, 128 * j:128 * j + 128],
                                start=True, stop=True, skip_group_check=True,
                                tile_position=(0, 0))
                            nc.tensor.matmul(
                                qk[:, 512:1024],
                                kt_sb[64:128, m, 128 * j:128 * j + 128],
                                qt_sb[64:128, 4 * sb:4 * sb + 4, 128 * j:128 * j + 128],
                                start=True, stop=True, skip_group_check=True,
                                tile_position=(64, 0))
                            ex = expp.tile([128, 1024], BF16)
                            nc.scalar.activation(ex[:, :], qk[:, :],
                                                 mybir.ActivationFunctionType.Exp,
                                                 bias=0.0, scale=0.125)
                            if m == 0 and pending is not None:
                                epilogue_finish(*pending)
                                pending = None
                            for i in range(2):
                                ct = 2 * m + i
                                nc.tensor.matmul(av[:, :],
                                                 v_sb[:, j, ct, :],
                                                 ex[:, 512 * i:512 * i + 512],
                                                 start=(ct == 0), stop=(ct == 15),
                                                 skip_group_check=True)
                            for _ in range(2):
                                try:
                                    next(gen)
                                except StopIteration:
                                    break
                        ds = smp.tile([1, 512], F32)
                        nc.vector.tensor_copy(ds[:, :], av[64:65, :])
                        rc = smp.tile([1, 512], F32)
                        nc.vector.reciprocal_approx_fast(rc[:, :], ds[:, :])
                        rcb = smp.tile([1, 512], BF16)
                        nc.vector.tensor_copy(rcb[:, :], rc[:, :])
                        bc = pbc.tile([64, 512], F32)
                        pending = (j, sb, av, rcb, bc)
                    # pending carries across group boundaries (the next
                    # group's op fillers don't read this group's last att2
                    # block until well after its deferred store lands); only
                    # the final block drains here.
                    if j == 3:
                        op3_h1a()
                        epilogue_finish(*pending)
                        pending = None
                    for _ in gen:
                        pass

                for _ in op3_rest():
                    pass

            wpv_cm.__exit__(None, None, None)
            qkv_cm.__exit__(None, None, None)
            att_cm.__exit__(None, None, None)

    nc.compile()
    return nc


def _make_runner(nc):
    """Build the jitted shard_map runner once (mirrors run_bass_via_pjrt)."""
    import jax
    import jax.numpy as jnp
    from jax.sharding import Mesh, PartitionSpec, NamedSharding
    from jax.experimental.shard_map import shard_map
    from concourse import bass2jax
    from concourse.bass2jax import _bass_exec_p, install_neuronx_cc_hook

    install_neuronx_cc_hook()

    part_tensor_name = nc.partition_id_tensor.name if nc.partition_id_tensor else None
    in_names = []
    out_names = []
    out_avals = []
    zero_shapes = []
    for alloc in nc.m.functions[0].allocations:
        if not isinstance(alloc, mybir.MemoryLocationSet):
            continue
        name = alloc.memorylocations[0].name
        if alloc.kind == "ExternalInput":
            if name != part_tensor_name:
                in_names.append(name)
        elif alloc.kind == "ExternalOutput":
            shape = tuple(alloc.tensor_shape)
            dtype = mybir.dt.np(alloc.dtype)
            out_names.append(name)
            out_avals.append(jax.core.ShapedArray(shape, dtype))
            zero_shapes.append((shape, dtype))
    n_params = len(in_names)
    all_names = list(in_names) + list(out_names)
    part_name = nc.partition_id_tensor.name if nc.partition_id_tensor else None
    if part_name is not None:
        all_names.append(part_name)

    def _body(*args):
        operands = list(args)
        if part_name is not None:
            operands.append(bass2jax.partition_id_tensor())
        outs = _bass_exec_p.bind(
            *operands,
            out_avals=tuple(out_avals),
            in_names=tuple(all_names),
            out_names=tuple(out_names),
            lowering_input_output_aliases=(),
            sim_require_finite=True,
            sim_require_nnan=True,
            nc=nc,
        )
        return tuple(outs)

    devices = jax.devices()[:N_CORES]
    mesh = Mesh(np.asarray(devices), ("core",))
    spec = PartitionSpec("core")
    n_outs = len(out_names)
    donate = tuple(range(n_params, n_params + n_outs))
    sharded = jax.jit(
        shard_map(_body, mesh=mesh,
                  in_specs=(spec,) * (n_params + n_outs),
                  out_specs=(spec,) * n_outs,
                  check_rep=False),
        donate_argnums=donate,
        keep_unused=True,
    )

    def _zeros():
        return tuple(
            jnp.zeros((N_CORES * s[0],) + tuple(s[1:]), dt)
            for s, dt in zero_shapes
        )

    zeros_fn = jax.jit(
        _zeros,
        out_shardings=tuple(NamedSharding(mesh, spec) for _ in zero_shapes),
    )

    return {
        "in_names": in_names,
        "out_names": out_names,
        "sharded": sharded,
        "zeros_fn": zeros_fn,
        "mesh": mesh,
        "spec": spec,
    }


def _tr_w(W):
    # [1024, 1024] -> [128, 8, 1024]: [p, kc, f] = W[f, 128*kc+p]
    return np.ascontiguousarray(W.T.reshape(8, 128, D).transpose(1, 0, 2))


def _prep_weights(Wq, bq, Wk, bk, Wv, bv, Wo, bo):
    bf = ml_dtypes.bfloat16
    wqt = _tr_w(np.asarray(Wq, np.float32)).astype(bf)
    wkt = _tr_w(np.asarray(Wk, np.float32)).astype(bf)
    wvt = _tr_w(np.asarray(Wv, np.float32)).astype(bf)
    wot = _tr_w(np.asarray(Wo, np.float32)).astype(bf)
    bqp = np.ascontiguousarray(np.asarray(bq, np.float32).reshape(8, 128).T)
    bqr = np.asarray(bq, np.float32).reshape(1, D).astype(bf)
    bkp = np.ascontiguousarray(np.asarray(bk, np.float32).reshape(8, 128).T)
    bkr = np.asarray(bk, np.float32).reshape(1, D).astype(bf)
    bv2 = np.asarray(bv, np.float32).reshape(1, D).astype(bf)
    return {"wqt": wqt, "wkt": wkt, "wvt": wvt, "wot": wot,
            "bqp": bqp, "bqr": bqr, "bkp": bkp, "bkr": bkr, "bv": bv2}


def _global_x(x_bf):
    # [4096, 1024] bf16 -> global [8*128, 8, 512]:
    # G[c*128+p, kc, r] = x[512*c + r, 128*kc + p]
    return np.ascontiguousarray(
        x_bf.reshape(8, 512, 8, 128).transpose(0, 3, 2, 1).reshape(1024, 8, 512))


def kernel(query, key, value, Wq, bq, Wk, bk, Wv, bv, Wo, bo):
    global LAST_EXEC_NS
    import jax
    from jax.sharding import NamedSharding

    if "nc" not in _CACHE:
        _install_ntff_hook()
        _CACHE["nc"] = _build()
        _CACHE["runner"] = _make_runner(_CACHE["nc"])
    nc = _CACHE["nc"]
    run = _CACHE["runner"]

    bf = ml_dtypes.bfloat16
    wkey = hash((np.asarray(Wq, np.float32).tobytes(),
                 np.asarray(Wk, np.float32).tobytes(),
                 np.asarray(Wv, np.float32).tobytes(),
                 np.asarray(Wo, np.float32).tobytes(),
                 np.asarray(bq, np.float32).tobytes(),
                 np.asarray(bk, np.float32).tobytes(),
                 np.asarray(bv, np.float32).tobytes()))
    if _CACHE.get("wkey") != wkey:
        wmap = _prep_weights(Wq, bq, Wk, bk, Wv, bv, Wo, bo)
        sharding = NamedSharding(run["mesh"], run["spec"])
        dev_w = {}
        for name, arr in wmap.items():
            g = np.concatenate([arr] * N_CORES, axis=0)
            dev_w[name] = jax.device_put(g, sharding)
        _CACHE["dev_w"] = dev_w
        _CACHE["wkey"] = wkey

    dev_w = _CACHE["dev_w"]
    sharding = NamedSharding(run["mesh"], run["spec"])

    def run_once():
        xq = _global_x(np.asarray(query, np.float32).reshape(4096, D).astype(bf))
        xk = _global_x(np.asarray(key, np.float32).reshape(4096, D).astype(bf))
        xv = _global_x(np.asarray(value, np.float32).reshape(4096, D).astype(bf))
        args = []
        for name in run["in_names"]:
            if name == "xqt":
                args.append(jax.device_put(xq, sharding))
            elif name == "xkt":
                args.append(jax.device_put(xk, sharding))
            elif name == "xvt":
                args.append(jax.device_put(xv, sharding))
            else:
                args.append(dev_w[name])
        zeros = run["zeros_fn"]()
        outs = run["sharded"](*args, *zeros)
        return {name: np.asarray(outs[i]) for i, name in enumerate(run["out_names"])}

    # warmup (first call compiles the wrapper executable)
    if "warm" not in _CACHE:
        run_once()
        _CACHE["warm"] = True

    t0 = time.perf_counter()
    res = run_once()
    wall_ns = int((time.perf_counter() - t0) * 1e9)
    _CACHE["wall_ns"] = wall_ns

    # Honest HW execution time: profile once via NTFF (device-side timing).
    if "hw_ns" not in _CACHE:
        _CACHE["hw_ns"] = _measure_hw_ns(query, key, value)
    LAST_EXEC_NS = _CACHE["hw_ns"] if _CACHE["hw_ns"] else wall_ns

    y = res["y"]  # [4096, 1024] bf16
    out = y.astype(np.float32) + np.asarray(bo, np.float32)[None, :]
    return out.reshape(2, 2048, D).astype(np.float32)


def _measure_hw_ns(query, key, value):
    """Run once under NTFF profiling; return on-device NEFF exec time (ns)."""
    try:
        from concourse.bass_utils import run_bass_kernel_spmd

        nc = _CACHE["nc"]
        bf = ml_dtypes.bfloat16
        xq = _global_x(np.asarray(query, np.float32).reshape(4096, D).astype(bf))
        xk = _global_x(np.asarray(key, np.float32).reshape(4096, D).astype(bf))
        xv = _global_x(np.asarray(value, np.float32).reshape(4096, D).astype(bf))
        dev_w = _CACHE["dev_w"]
        # per-core input maps (host copies)
        in_maps = []
        for c in range(N_CORES):
            m = {"xqt": xq[128 * c:128 * c + 128],
                 "xkt": xk[128 * c:128 * c + 128],
                 "xvt": xv[128 * c:128 * c + 128]}
            for name, arr in dev_w.items():
                full = np.asarray(arr)
                per = full.shape[0] // N_CORES
                m[name] = full[per * c:per * c + per]
            in_maps.append(m)
        import shutil
        tdir = "/tmp/ntff_last"
        shutil.rmtree(tdir, ignore_errors=True)
        os.makedirs(tdir, exist_ok=True)
        rr = run_bass_kernel_spmd(nc, in_maps, list(range(N_CORES)), trace=True,
                                  tmpdir=tdir)
        return rr.exec_time_ns
    except Exception:
        return None
